# revision 31
# baseline (speedup 1.0000x reference)
"""Trainium2 Bass kernel for nn_Attn_6545530159401.

Computation (reference):
    enc  = encoder_outputs.transpose(1,0,2)            # (B,T,H)
    cat  = concat([hidden broadcast, enc], -1)         # (B,T,2H)
    en   = tanh(cat @ W_attn.T + b_attn)               # (B,T,H)
    sc   = en @ v                                      # (B,T)
    out  = softmax(sc, axis=1)[:, None, :]             # (B,1,T)

Split W_attn = [W_h | W_e] (each (H,H)):
    q[b]     = hidden[b] @ W_h.T + b_attn              # (B,H) host-precomputed
    E[b,t]   = enc[b,t] @ W_e.T                        # the big matmul
    sc[b,t]  = sum_o v[o] * tanh(q[b,o] + E[b,t,o])

Sharding: data-parallel over B across 8 NeuronCores (4 batches/core),
no collectives. Per-core pipeline (o-chunks on PSUM partitions so q can
ride the ACT bias port):
    mains   PE: E-psum (128 o, 1024 rows) via fp8e4 DoubleRow matmuls
            (2 fp8 weights/cell -> K=256 per matmul, 2 matmuls per
            512-col block; operands pre-scaled on host, descale rides
            the ACT scale immediate)
    tanh    ACT: tanh(E/16384 + q[b, o-chunk]) via per-partition bias
            (the serial ACT pass over all 4.2M elements is the kernel's
            critical path; everything else hides under it)
    scores  PE: per (s,h2) four col-tiled v-window matmuls run
            concurrently in the four 32-column array strips
            (tile_position=(0,32*o)), each contracting one o-chunk's
            tanh tile into rows 32*o + (2s+h2) of a zero-initialized
            (128,512) psum bank; a final DVE copy + one select-matmul
            folds the four strips into the (16,512) score layout
    softmax ACT exp only; the row sums and division (0.0002% of FLOPs)
            happen in the host gather step, cutting the serial
            cross-engine tail hops. No max-subtraction (|s| <= ||v||_1).
Head: o0 weight slice + rest + consts ride SWDGE (gpsimd clears its
preamble ~1.3us before the HWDGE rings, and e1..e7 queue FIFO behind
them on the same ring so they cannot steal bandwidth); enc superblock 0
is four SEPARATE 128KB tiles split across both HWDGE rings so each of
the first matmuls waits only on its own slice. Warmup matmuls on a
zeroed tile hold the PE clock gate (HAM) at 2.4 GHz through the head.
"""

import numpy as np
from contextlib import ExitStack

import concourse.bass as bass
import concourse.tile as tile
from concourse import bacc, mybir
import concourse.bass_utils as bass_utils

T, B, H = 2048, 32, 512
NCORES = 8
NB = B // NCORES        # 4 local batches per core
ROWS = NB * T           # 8192 rows per core
P = 128
KP = 2                  # DoubleRow contraction chunks (256 each)
OC = H // P             # 4 output chunks
SUP = 1024              # columns per E-psum tile (2 PSUM banks)
NSUP = ROWS // SUP      # 8
BLK = 512               # matmul moving-dim limit (one PSUM bank)
SE = 32.0               # fp8 scale on enc  (|enc|*32  < 240)
SW = 512.0              # fp8 scale on W_e  (|W|*512 <= 16)
WB = OC * KP * 2 * P    # wet8 bytes/partition (2048)
CB = WB + 512           # head param bytes/partition: wet8 + consts
F32 = mybir.dt.float32
F16 = mybir.dt.float16
F8 = mybir.dt.float8e4
AF = mybir.ActivationFunctionType
DR = mybir.MatmulPerfMode.DoubleRow
F8NP = mybir.dt.np(F8)


def _build():
    nc = bacc.Bacc(
        "TRN2", target_bir_lowering=False, debug=False, num_devices=NCORES
    )
    # head[p, 0:2048]    = wet8[p, o, kp, i, m]
    #                    = W_e^T[kp*256 + i*128 + p, o*128 + m] * SW (fp8)
    # head[p, 2048:2560] = consts row p viewed as fp8 bytes (bitcast f32):
    #   f32 cols  0:16  qrep[p, o*NB+b] = q[b, o*128+p] (host-computed)
    #   f32 cols 40:48  sel16 as f16[16]: sel[32o+c, c] = 1 (strip fold)
    #   f32 cols 64:128 vwin: 4 chunks of 32 f16 cols, chunk o has
    #                   v[o*128+p] at f16-col 128+32*o+15 (sliding lhsT)
    head = nc.declare_dram_parameter("head", [P, CB], F8, isOutput=False)
    # enc8[p, s, kp, t2, i, t] = enc^T[kp*256+i*128+p, s*1024+t2*512+t]*SE
    enc8 = nc.declare_dram_parameter("enc8", [P, NSUP * KP * 2 * 2 * BLK], F8,
                                     isOutput=False)
    # unnormalized softmax: rows 4b+j hold exp(scores) of batch b,
    # t-slice j*512 (host sums and divides)
    out = nc.declare_dram_parameter("out", [16, BLK], F32, isOutput=True)

    with tile.TileContext(nc) as tc, ExitStack() as ctx:
        const_pool = ctx.enter_context(tc.tile_pool(name="const", bufs=1))
        enc_pool = ctx.enter_context(tc.tile_pool(name="enc", bufs=1))
        tanh_pool = ctx.enter_context(tc.tile_pool(name="tanh", bufs=12))
        sm_pool = ctx.enter_context(tc.tile_pool(name="sm", bufs=1))
        psE_pool = ctx.enter_context(tc.tile_pool(name="psE", bufs=7, space="PSUM"))
        psS_pool = ctx.enter_context(tc.tile_pool(name="psS", bufs=1, space="PSUM"))

        # weights + consts via SWDGE (the gpsimd queue clears its
        # preamble ~1.3us before the HWDGE rings); the o=0 weight slice
        # goes first so the very first matmul can start early, and the
        # e1..e7 stream queues FIFO behind these on the same ring.
        h_sb = const_pool.tile([P, CB], F8, tag="head")
        nc.gpsimd.dma_start(h_sb[:, 0:512], head[:, 0:512])          # w8 o=0
        nc.gpsimd.dma_start(h_sb[:, 512:CB], head[:, 512:CB])        # rest
        c_sb = h_sb[:, WB:CB].bitcast(F32)   # (P, 128) f32 view
        c16 = h_sb[:, WB:CB].bitcast(F16)    # (P, 256) f16 view
        q_sb = c_sb[:, 0:16]
        sel16_sb = c16[:, 80:96]
        vwin_sb = [c16[:, 128 + 32 * o : 159 + 32 * o] for o in range(OC)]
        # lhsT AP per (o, kp): [128, 2, 128], pair-stride 128 elements
        w8 = h_sb[:, 0:WB].rearrange("p (o k i m) -> p o k i m", o=OC, k=KP, i=2)

        # enc superblock 0 as four separate (kp, t-half) tiles split
        # across both HWDGE rings: each of the first four matmuls waits
        # only on its own 128KB slice.
        SUPB = KP * 2 * 2 * BLK  # fp8 bytes per partition per superblock
        e0t = [[None] * 2 for _ in range(KP)]
        for t2 in range(2):
            for kp in range(KP):
                tl = enc_pool.tile([P, 2 * BLK], F8, tag=f"e0_{kp}_{t2}")
                off = kp * 2 * 2 * BLK + t2 * 2 * BLK
                eng = nc.sync if kp == 0 else nc.scalar
                eng.dma_start(tl[:], enc8[:, off : off + 2 * BLK])
                e0t[kp][t2] = tl

        # PE warmup: matmuls on a zeroed scratch tile run while the first
        # DMAs are still in flight, so the HAM clock gate is already
        # released (2.4 GHz) when real matmuls start. The memset rides
        # DVE (idle, clears its preamble early) and the warmup target is
        # the score bank, which the later DVE memset re-zeroes anyway.
        warm = const_pool.tile([P, BLK], F16, tag="warm")
        nc.vector.memset(warm[:], 0.0)
        psS = psS_pool.tile([P, BLK], F32, tag="s", name="psS")
        for _ in range(5):
            nc.tensor.matmul(
                psS[:], lhsT=warm[:, 0:P], rhs=warm[:], start=True, stop=True
            )

        # later superblocks stream whole via SWDGE, FIFO behind the head
        enc_sb = [None] * NSUP
        for s in range(1, NSUP):
            e = enc_pool.tile([P, SUPB], F8, tag=f"e{s}", name=f"e{s}")
            nc.gpsimd.dma_start(e[:], enc8[:, s * SUPB : (s + 1) * SUPB])
            enc_sb[s] = e

        # score accumulator: rows 32*o + (2s+h2) collect o-chunk partial
        # scores of (batch s//2, t-slice (s%2)*1024 + h2*512); the four
        # o-strips are summed by the sel16 matmul after the main loop.
        # DVE-zeroed (after the warmup matmuls) so the never-written rows
        # contract to 0.
        nc.vector.memset(psS[:], 0.0)

        def emit_vdots(ths, s, only_h2=None):
            # four col-tiled matmuls per h2 run concurrently in the four
            # 32-column PE strips; one 4-pack (~1 matmul span) fits the
            # per-group PE slack, so the two packs are emitted at
            # different o-slots instead of back-to-back
            for h2 in range(SUP // BLK):
                if only_h2 is not None and h2 != only_h2:
                    continue
                c = 2 * s + h2
                for o in range(OC):
                    nc.tensor.matmul(
                        psS[32 * o : 32 * o + 16, :],
                        lhsT=vwin_sb[o][:, 15 - c : 31 - c],
                        rhs=ths[o][:, h2 * BLK : (h2 + 1) * BLK],
                        start=False,
                        stop=False,
                        tile_position=(0, 32 * o),
                        skip_group_check=True,
                    )

        pending = None
        for s in range(NSUP):
            b = s // 2
            last_sup = s == NSUP - 1
            if s == 0:
                # rhs AP per (kp, h2): [128, 2, 512] within one 128KB tile
                rhs_ap = lambda kp, h2: e0t[kp][h2][:].rearrange(
                    "p (i t) -> p i t", i=2
                )
            else:
                es = enc_sb[s][:].rearrange(
                    "p (k t2 i t) -> p k t2 i t", k=KP, t2=2, i=2
                )
                rhs_ap = lambda kp, h2: es[:, kp, h2]
            ths = []
            for o in range(OC):
                # one single-bank psum tile + one N=512 tanh per h2 half:
                # with 7 rotating banks the psum-recycle chain (tanh
                # completion -> cross-engine sem -> matmul group) never
                # gates the ACT stream, which runs back-to-back
                th = tanh_pool.tile([P, SUP], F16, tag="tanh")
                for h2 in range(SUP // BLK):
                    psE = psE_pool.tile([P, BLK], F32, tag="E")
                    for kp in range(KP):
                        nc.tensor.matmul(
                            psE[:],
                            lhsT=w8[:, o, kp],
                            rhs=rhs_ap(kp, h2),
                            start=(kp == 0),
                            stop=(kp == KP - 1),
                            perf_mode=DR,
                        )
                    nc.scalar.activation(
                        th[:, h2 * BLK : (h2 + 1) * BLK],
                        psE[:],
                        AF.Tanh,
                        bias=q_sb[:, o * NB + b : o * NB + b + 1],
                        scale=1.0 / (SE * SW),
                    )
                ths.append(th)
                if pending is not None and o in (1, 3):
                    emit_vdots(*pending, only_h2=o // 2)
                    if o == 3:
                        pending = None
                if last_sup and o > 0:
                    # eager per-o v-dots so only o=3's trail the last tanh
                    oo = o - 1
                    for h2 in range(SUP // BLK):
                        c = 2 * s + h2
                        nc.tensor.matmul(
                            psS[32 * oo : 32 * oo + 16, :],
                            lhsT=vwin_sb[oo][:, 15 - c : 31 - c],
                            rhs=ths[oo][:, h2 * BLK : (h2 + 1) * BLK],
                            start=False,
                            stop=False,
                            tile_position=(0, 32 * oo),
                            skip_group_check=True,
                        )
            if not last_sup:
                pending = (ths, s)
        for h2 in range(SUP // BLK):
            c = 2 * (NSUP - 1) + h2
            nc.tensor.matmul(
                psS[96:112, :],
                lhsT=vwin_sb[3][:, 15 - c : 31 - c],
                rhs=ths[3][:, h2 * BLK : (h2 + 1) * BLK],
                start=False,
                stop=False,
                tile_position=(0, 96),
                skip_group_check=True,
            )

        # fold the four o-strips: copy psum->sbuf f16, contract with the
        # sel16 lhsT (sel[32o+c, c] = 1) into the (16,512) score tile
        sc_sb = sm_pool.tile([P, BLK], F16, tag="sc")
        nc.vector.tensor_copy(sc_sb[:], psS[:])
        # reuse psS's bank: the copy has drained it, so the folded scores
        # can land in its first 16 partitions
        psS16 = psS[0:16, :]
        nc.tensor.matmul(
            psS16, lhsT=sel16_sb[:], rhs=sc_sb[:], start=True, stop=True,
            skip_group_check=True,
        )
        ex16 = sm_pool.tile([16, BLK], F32, tag="ex16")
        nc.scalar.activation(ex16[:], psS16, AF.Exp)
        nc.sync.dma_start(out[:, :], ex16[:])

    nc.compile()
    return nc


_NC = None


def _get_nc():
    global _NC
    if _NC is None:
        _NC = _build()
    return _NC


def _to_f8(x):
    return np.asarray(np.clip(x, -240.0, 240.0), dtype=F8NP)


def _shard_inputs(hidden, encoder_outputs, W_attn, b_attn, v):
    hidden = np.asarray(hidden, dtype=np.float32)
    encoder_outputs = np.asarray(encoder_outputs, dtype=np.float32)
    W_attn = np.asarray(W_attn, dtype=np.float32)
    b_attn = np.asarray(b_attn, dtype=np.float32)
    v = np.asarray(v, dtype=np.float32)

    # wet8[p, o, kp, i, m] = W_e^T[kp*256 + i*128 + p, o*128 + m] * SW
    wet = (W_attn[:, H:].T * SW).reshape(KP, 2, P, OC, P)  # [kp,i,p,o,m]
    wet8 = _to_f8(
        np.ascontiguousarray(wet.transpose(2, 3, 0, 1, 4)).reshape(P, -1)
    )

    # q[b, o] = hidden[b] @ W_h.T + b_attn, computed on host (tiny)
    q = hidden[0] @ W_attn[:, :H].T + b_attn  # (B, H)

    # packed constant block, f32 view (P, 128) / f16 view (P, 256)
    consts = np.zeros((P, 128), dtype=np.float32)
    c16 = consts.view(np.float16)  # (P, 256)
    for o in range(OC):
        for c in range(16):
            c16[32 * o + c, 80 + c] = np.float16(1.0)  # sel16 strip fold
    vrT = v.reshape(OC, P).T.astype(np.float16)  # (P, OC)
    for o in range(OC):
        c16[:, 128 + 32 * o + 15] = vrT[:, o]  # vwin sliding windows

    # (H, B, T) so per-core slices are cheap views before the copy
    enc_hbt = np.transpose(encoder_outputs, (2, 1, 0))
    in_maps = []
    for c in range(NCORES):
        b0 = c * NB
        # enc8[p, s, kp, t2, i, t] = enc^T[kp*256+i*128+p, s*1024+t2*512+t]
        encT = np.ascontiguousarray(
            enc_hbt[:, b0 : b0 + NB, :], dtype=np.float32
        ).reshape(KP, 2, P, NSUP, 2, BLK)  # [kp, i, p, s, t2, t]
        enc8 = _to_f8(
            np.ascontiguousarray(
                encT.transpose(2, 3, 0, 4, 1, 5) * SE
            ).reshape(P, -1)
        )
        cc = consts.copy()
        # qrep[p, o*NB+b] = q[b0+b, o*128+p]
        qc = q[b0 : b0 + NB].T.reshape(OC, P, NB)  # [o, p, b]
        cc[:, 0:16] = qc.transpose(1, 0, 2).reshape(P, OC * NB)
        headarr = np.concatenate(
            [wet8, cc.view(F8NP)], axis=1
        )  # (P, CB) fp8 bytes
        in_maps.append({"head": headarr, "enc8": enc8})
    return in_maps


def kernel(hidden, encoder_outputs, W_attn, b_attn, v):
    nc = _get_nc()
    in_maps = _shard_inputs(hidden, encoder_outputs, W_attn, b_attn, v)
    res = bass_utils.run_bass_kernel_spmd(
        nc, in_maps, core_ids=list(range(NCORES))
    )
    outs = []
    for c in range(NCORES):
        ex = np.asarray(res.results[c]["out"], dtype=np.float64).reshape(NB, T)
        outs.append(ex / ex.sum(axis=1, keepdims=True))
    full = np.concatenate(outs, axis=0)  # (B, T)
    return full[:, None, :].astype(np.float32)  # (B, 1, T)


# revision 32
# speedup vs baseline: 1.0609x; 1.0609x over previous
"""Trainium2 Bass kernel for nn_Attn_6545530159401.

Computation (reference):
    enc  = encoder_outputs.transpose(1,0,2)            # (B,T,H)
    cat  = concat([hidden broadcast, enc], -1)         # (B,T,2H)
    en   = tanh(cat @ W_attn.T + b_attn)               # (B,T,H)
    sc   = en @ v                                      # (B,T)
    out  = softmax(sc, axis=1)[:, None, :]             # (B,1,T)

Split W_attn = [W_h | W_e] (each (H,H)):
    q[b]     = hidden[b] @ W_h.T + b_attn              # (B,H) host-precomputed
    E[b,t]   = enc[b,t] @ W_e.T                        # the big matmul
    sc[b,t]  = sum_o v[o] * tanh(q[b,o] + E[b,t,o])

Sharding: data-parallel over B across 8 NeuronCores (4 batches/core),
no collectives. Per-core pipeline (o-chunks on PSUM partitions so q can
ride the ACT bias port):
    mains   PE: E-psum (128 o, 1024 rows) via fp8e4 DoubleRow matmuls
            (2 fp8 weights/cell -> K=256 per matmul, 2 matmuls per
            512-col block; operands pre-scaled on host, descale rides
            the ACT scale immediate)
    tanh    ACT: tanh(E/16384 + q[b, o-chunk]) via per-partition bias
            (the serial ACT pass over all 4.2M elements is the kernel's
            critical path; everything else hides under it)
    scores  PE: per (s,h2) four col-tiled v-window matmuls run
            concurrently in the four 32-column array strips
            (tile_position=(0,32*o)), each contracting one o-chunk's
            tanh tile into rows 32*o + (2s+h2) of a zero-initialized
            (128,512) psum bank; a final DVE copy + one select-matmul
            folds the four strips into the (16,512) score layout
    softmax ACT exp only; the row sums and division (0.0002% of FLOPs)
            happen in the host gather step, cutting the serial
            cross-engine tail hops. No max-subtraction (|s| <= ||v||_1).
Head: o0 weight slice + rest + consts ride SWDGE (gpsimd clears its
preamble ~1.3us before the HWDGE rings, and e1..e7 queue FIFO behind
them on the same ring so they cannot steal bandwidth); enc superblock 0
is four SEPARATE 128KB tiles split across both HWDGE rings so each of
the first matmuls waits only on its own slice. Warmup matmuls on a
zeroed tile hold the PE clock gate (HAM) at 2.4 GHz through the head.
"""

import numpy as np
from contextlib import ExitStack

import concourse.bass as bass
import concourse.tile as tile
from concourse import bacc, mybir
import concourse.bass_utils as bass_utils

T, B, H = 2048, 32, 512
NCORES = 8
NB = B // NCORES        # 4 local batches per core
ROWS = NB * T           # 8192 rows per core
P = 128
KP = 2                  # DoubleRow contraction chunks (256 each)
OC = H // P             # 4 output chunks
SUP = 1024              # columns per E-psum tile (2 PSUM banks)
NSUP = ROWS // SUP      # 8
BLK = 512               # matmul moving-dim limit (one PSUM bank)
SE = 32.0               # fp8 scale on enc  (|enc|*32  < 240)
SW = 512.0              # fp8 scale on W_e  (|W|*512 <= 16)
WB = OC * KP * 2 * P    # wet8 bytes/partition (2048)
CB = WB + 512           # head param bytes/partition: wet8 + consts
F32 = mybir.dt.float32
F16 = mybir.dt.float16
F8 = mybir.dt.float8e4
AF = mybir.ActivationFunctionType
DR = mybir.MatmulPerfMode.DoubleRow
F8NP = mybir.dt.np(F8)


def _build():
    nc = bacc.Bacc(
        "TRN2", target_bir_lowering=False, debug=False, num_devices=NCORES
    )
    # head[p, 0:2048]    = wet8[p, o, kp, i, m]
    #                    = W_e^T[kp*256 + i*128 + p, o*128 + m] * SW (fp8)
    # head[p, 2048:2560] = consts row p viewed as fp8 bytes (bitcast f32):
    #   f32 cols  0:16  qrep[p, o*NB+b] = q[b, o*128+p] (host-computed)
    #   f32 cols 40:48  sel16 as f16[16]: sel[32o+c, c] = 1 (strip fold)
    #   f32 cols 64:128 vwin: 4 chunks of 32 f16 cols, chunk o has
    #                   v[o*128+p] at f16-col 128+32*o+15 (sliding lhsT)
    head = nc.declare_dram_parameter("head", [P, CB], F8, isOutput=False)
    # enc8[p, s, kp, t2, i, t] = enc^T[kp*256+i*128+p, s*1024+t2*512+t]*SE
    enc8 = nc.declare_dram_parameter("enc8", [P, NSUP * KP * 2 * 2 * BLK], F8,
                                     isOutput=False)
    # unnormalized softmax: rows 4b+j hold exp(scores) of batch b,
    # t-slice j*512 (host sums and divides)
    out = nc.declare_dram_parameter("out", [16, BLK], F32, isOutput=True)

    with tile.TileContext(nc) as tc, ExitStack() as ctx:
        const_pool = ctx.enter_context(tc.tile_pool(name="const", bufs=1))
        enc_pool = ctx.enter_context(tc.tile_pool(name="enc", bufs=1))
        tanh_pool = ctx.enter_context(tc.tile_pool(name="tanh", bufs=8))
        sm_pool = ctx.enter_context(tc.tile_pool(name="sm", bufs=1))
        psE_pool = ctx.enter_context(tc.tile_pool(name="psE", bufs=7, space="PSUM"))
        psS_pool = ctx.enter_context(tc.tile_pool(name="psS", bufs=1, space="PSUM"))

        # weights + consts via SWDGE (the gpsimd queue clears its
        # preamble ~1.3us before the HWDGE rings); the o=0 weight slice
        # goes first so the very first matmul can start early, and the
        # e1..e7 stream queues FIFO behind these on the same ring.
        h_sb = const_pool.tile([P, CB], F8, tag="head")
        nc.gpsimd.dma_start(h_sb[:, 0:512], head[:, 0:512])          # w8 o=0
        nc.gpsimd.dma_start(h_sb[:, 512:CB], head[:, 512:CB])        # rest
        c_sb = h_sb[:, WB:CB].bitcast(F32)   # (P, 128) f32 view
        c16 = h_sb[:, WB:CB].bitcast(F16)    # (P, 256) f16 view
        q_sb = c_sb[:, 0:16]
        sel16_sb = c16[:, 80:96]
        vwin_sb = [c16[:, 128 + 32 * o : 159 + 32 * o] for o in range(OC)]
        # lhsT AP per (o, kp): [128, 2, 128], pair-stride 128 elements
        w8 = h_sb[:, 0:WB].rearrange("p (o k i m) -> p o k i m", o=OC, k=KP, i=2)

        # enc superblock 0 as four separate (kp, t-half) tiles split
        # across both HWDGE rings: each of the first four matmuls waits
        # only on its own 128KB slice.
        SUPB = KP * 2 * 2 * BLK  # fp8 bytes per partition per superblock
        e0t = [[None] * 2 for _ in range(KP)]
        for t2 in range(2):
            for kp in range(KP):
                tl = enc_pool.tile([P, 2 * BLK], F8, tag=f"e0_{kp}_{t2}")
                off = kp * 2 * 2 * BLK + t2 * 2 * BLK
                eng = nc.sync if kp == 0 else nc.scalar
                eng.dma_start(tl[:], enc8[:, off : off + 2 * BLK])
                e0t[kp][t2] = tl

        # PE warmup: matmuls on a zeroed scratch tile run while the first
        # DMAs are still in flight, so the HAM clock gate is already
        # released (2.4 GHz) when real matmuls start. The memset rides
        # DVE (idle, clears its preamble early) and the warmup target is
        # the score bank, which the later DVE memset re-zeroes anyway.
        warm = const_pool.tile([P, BLK], F16, tag="warm")
        nc.vector.memset(warm[:], 0.0)
        psS = psS_pool.tile([P, BLK], F32, tag="s", name="psS")
        for _ in range(5):
            nc.tensor.matmul(
                psS[:], lhsT=warm[:, 0:P], rhs=warm[:], start=True, stop=True
            )

        # later superblocks stream whole via SWDGE, FIFO behind the head
        enc_sb = [None] * NSUP
        for s in range(1, NSUP):
            e = enc_pool.tile([P, SUPB], F8, tag=f"e{s}", name=f"e{s}")
            nc.gpsimd.dma_start(e[:], enc8[:, s * SUPB : (s + 1) * SUPB])
            enc_sb[s] = e

        # score accumulator: rows 32*o + (2s+h2) collect o-chunk partial
        # scores of (batch s//2, t-slice (s%2)*1024 + h2*512); the four
        # o-strips are summed by the sel16 matmul after the main loop.
        # DVE-zeroed (after the warmup matmuls) so the never-written rows
        # contract to 0.
        nc.vector.memset(psS[:], 0.0)

        def emit_vdots(ths, s):
            # four col-tiled matmuls per h2 run concurrently in the four
            # 32-column PE strips, one per o-chunk
            for h2 in range(SUP // BLK):
                c = 2 * s + h2
                for o in range(OC):
                    nc.tensor.matmul(
                        psS[32 * o : 32 * o + 16, :],
                        lhsT=vwin_sb[o][:, 15 - c : 31 - c],
                        rhs=ths[o][:, h2 * BLK : (h2 + 1) * BLK],
                        start=False,
                        stop=False,
                        tile_position=(0, 32 * o),
                        skip_group_check=True,
                    )

        pending = None
        for s in range(NSUP):
            b = s // 2
            last_sup = s == NSUP - 1
            if s == 0:
                # rhs AP per (kp, h2): [128, 2, 512] within one 128KB tile
                rhs_ap = lambda kp, h2: e0t[kp][h2][:].rearrange(
                    "p (i t) -> p i t", i=2
                )
            else:
                es = enc_sb[s][:].rearrange(
                    "p (k t2 i t) -> p k t2 i t", k=KP, t2=2, i=2
                )
                rhs_ap = lambda kp, h2: es[:, kp, h2]
            ths = []
            for o in range(OC):
                # one single-bank psum tile + one N=512 tanh per h2 half:
                # with 7 rotating banks the psum-recycle chain (tanh
                # completion -> cross-engine sem -> matmul group) never
                # gates the ACT stream, which runs back-to-back
                th = tanh_pool.tile([P, SUP], F16, tag="tanh")
                for h2 in range(SUP // BLK):
                    psE = psE_pool.tile([P, BLK], F32, tag="E")
                    for kp in range(KP):
                        nc.tensor.matmul(
                            psE[:],
                            lhsT=w8[:, o, kp],
                            rhs=rhs_ap(kp, h2),
                            start=(kp == 0),
                            stop=(kp == KP - 1),
                            perf_mode=DR,
                        )
                    nc.scalar.activation(
                        th[:, h2 * BLK : (h2 + 1) * BLK],
                        psE[:],
                        AF.Tanh,
                        bias=q_sb[:, o * NB + b : o * NB + b + 1],
                        scale=1.0 / (SE * SW),
                    )
                ths.append(th)
                if pending is not None and o == 1:
                    emit_vdots(*pending)
                    pending = None
                if last_sup and o > 0:
                    # eager per-o v-dots so only o=3's trail the last tanh
                    oo = o - 1
                    for h2 in range(SUP // BLK):
                        c = 2 * s + h2
                        nc.tensor.matmul(
                            psS[32 * oo : 32 * oo + 16, :],
                            lhsT=vwin_sb[oo][:, 15 - c : 31 - c],
                            rhs=ths[oo][:, h2 * BLK : (h2 + 1) * BLK],
                            start=False,
                            stop=False,
                            tile_position=(0, 32 * oo),
                            skip_group_check=True,
                        )
            if not last_sup:
                pending = (ths, s)
        for h2 in range(SUP // BLK):
            c = 2 * (NSUP - 1) + h2
            nc.tensor.matmul(
                psS[96:112, :],
                lhsT=vwin_sb[3][:, 15 - c : 31 - c],
                rhs=ths[3][:, h2 * BLK : (h2 + 1) * BLK],
                start=False,
                stop=False,
                tile_position=(0, 96),
                skip_group_check=True,
            )

        # fold the four o-strips: copy psum->sbuf f16, contract with the
        # sel16 lhsT (sel[32o+c, c] = 1) into the (16,512) score tile
        sc_sb = sm_pool.tile([P, BLK], F16, tag="sc")
        nc.vector.tensor_copy(sc_sb[:], psS[:])
        # reuse psS's bank: the copy has drained it, so the folded scores
        # can land in its first 16 partitions
        psS16 = psS[0:16, :]
        nc.tensor.matmul(
            psS16, lhsT=sel16_sb[:], rhs=sc_sb[:], start=True, stop=True,
            skip_group_check=True,
        )
        ex16 = sm_pool.tile([16, BLK], F32, tag="ex16")
        nc.scalar.activation(ex16[:], psS16, AF.Exp)
        nc.sync.dma_start(out[:, :], ex16[:])

    nc.compile()
    return nc


_NC = None


def _get_nc():
    global _NC
    if _NC is None:
        _NC = _build()
    return _NC


def _to_f8(x):
    return np.asarray(np.clip(x, -240.0, 240.0), dtype=F8NP)


def _shard_inputs(hidden, encoder_outputs, W_attn, b_attn, v):
    hidden = np.asarray(hidden, dtype=np.float32)
    encoder_outputs = np.asarray(encoder_outputs, dtype=np.float32)
    W_attn = np.asarray(W_attn, dtype=np.float32)
    b_attn = np.asarray(b_attn, dtype=np.float32)
    v = np.asarray(v, dtype=np.float32)

    # wet8[p, o, kp, i, m] = W_e^T[kp*256 + i*128 + p, o*128 + m] * SW
    wet = (W_attn[:, H:].T * SW).reshape(KP, 2, P, OC, P)  # [kp,i,p,o,m]
    wet8 = _to_f8(
        np.ascontiguousarray(wet.transpose(2, 3, 0, 1, 4)).reshape(P, -1)
    )

    # q[b, o] = hidden[b] @ W_h.T + b_attn, computed on host (tiny)
    q = hidden[0] @ W_attn[:, :H].T + b_attn  # (B, H)

    # packed constant block, f32 view (P, 128) / f16 view (P, 256)
    consts = np.zeros((P, 128), dtype=np.float32)
    c16 = consts.view(np.float16)  # (P, 256)
    for o in range(OC):
        for c in range(16):
            c16[32 * o + c, 80 + c] = np.float16(1.0)  # sel16 strip fold
    vrT = v.reshape(OC, P).T.astype(np.float16)  # (P, OC)
    for o in range(OC):
        c16[:, 128 + 32 * o + 15] = vrT[:, o]  # vwin sliding windows

    # (H, B, T) so per-core slices are cheap views before the copy
    enc_hbt = np.transpose(encoder_outputs, (2, 1, 0))
    in_maps = []
    for c in range(NCORES):
        b0 = c * NB
        # enc8[p, s, kp, t2, i, t] = enc^T[kp*256+i*128+p, s*1024+t2*512+t]
        encT = np.ascontiguousarray(
            enc_hbt[:, b0 : b0 + NB, :], dtype=np.float32
        ).reshape(KP, 2, P, NSUP, 2, BLK)  # [kp, i, p, s, t2, t]
        enc8 = _to_f8(
            np.ascontiguousarray(
                encT.transpose(2, 3, 0, 4, 1, 5) * SE
            ).reshape(P, -1)
        )
        cc = consts.copy()
        # qrep[p, o*NB+b] = q[b0+b, o*128+p]
        qc = q[b0 : b0 + NB].T.reshape(OC, P, NB)  # [o, p, b]
        cc[:, 0:16] = qc.transpose(1, 0, 2).reshape(P, OC * NB)
        headarr = np.concatenate(
            [wet8, cc.view(F8NP)], axis=1
        )  # (P, CB) fp8 bytes
        in_maps.append({"head": headarr, "enc8": enc8})
    return in_maps


def kernel(hidden, encoder_outputs, W_attn, b_attn, v):
    nc = _get_nc()
    in_maps = _shard_inputs(hidden, encoder_outputs, W_attn, b_attn, v)
    res = bass_utils.run_bass_kernel_spmd(
        nc, in_maps, core_ids=list(range(NCORES))
    )
    outs = []
    for c in range(NCORES):
        ex = np.asarray(res.results[c]["out"], dtype=np.float64).reshape(NB, T)
        outs.append(ex / ex.sum(axis=1, keepdims=True))
    full = np.concatenate(outs, axis=0)  # (B, T)
    return full[:, None, :].astype(np.float32)  # (B, 1, T)
